# revision 1
# baseline (speedup 1.0000x reference)
"""Low-rank attention kernel for Trainium2, distributed over 8 NeuronCores.

Math (per batch b):
    u  = q @ Wu            [N, R]
    vp = k @ Wv            [N, R]
    S  = u @ vp.T / sqrt(R)
    out = softmax(S) @ v   [N, D]

Shapes: B=4, N=4096, D=1024, R=32.

Sharding: data-parallel over batch x row-halves -> 8 shards. Core c handles
batch b = c // 2, rows [h*2048, (h+1)*2048) with h = c % 2. Each core gets its
q-shard and the full k/v for its batch. q/k are fed pre-transposed ([D, n]
layout) so every matmul contraction lands on the partition axis with no
on-device transposes.

Per-core device kernel (all matmuls in float32r: full PE rate, ~1e-4 rel err):
  1. uT[R, 2048]  = sum_d Wu[d, :].T qT[d, :]   (K=128 d-tiles, PSUM accum)
     vpT[R, 4096] = sum_d Wv[d, :].T kT[d, :]
  2. flash-style main loop over n-chunks of 256 rows:
       for each m-tile (128 cols): scoresT[m128, n256] = vpT_tile.T @ uT_chunk
       expT = Exp(scoresT / sqrt(R))                       (ScalarE, PSUM->SBUF)
       out_acc[n128, d512] += expT_tile.T @ v_tile         (PSUM accum over m)
       sum_acc[n128, 1]    += expT_tile.T @ ones
     out = out_acc * (1 / sum_acc)   (softmax normalization folded at the end)
"""

import numpy as np

B, N, D, R = 4, 4096, 1024, 32
NLOC = N // 2            # rows per core
RSCALE = float(1.0 / np.sqrt(np.float32(R)))

N_CHUNK = 256            # rows of scores computed per PSUM round
M_TILE = 128             # contraction tile for the AV matmul
D_HALF = 512             # PSUM bank width in fp32

LAST_RESULT = None       # test.py reads exec_time_ns etc. from here


def _build():
    from concourse import bacc, mybir
    from concourse.tile import TileContext

    f32 = mybir.dt.float32
    f32r = mybir.dt.float32r
    f16 = mybir.dt.float16
    EXP = mybir.ActivationFunctionType.Exp
    COPY = mybir.ActivationFunctionType.Copy

    nc = bacc.Bacc("TRN2", target_bir_lowering=False)

    qT = nc.dram_tensor("qT", [D, NLOC], f32r, kind="ExternalInput")
    kT = nc.dram_tensor("kT", [D, N], f32r, kind="ExternalInput")
    v = nc.dram_tensor("v", [N, D], f16, kind="ExternalInput")
    wu = nc.dram_tensor("wu", [D, R], f32r, kind="ExternalInput")
    wv = nc.dram_tensor("wv", [D, R], f32r, kind="ExternalInput")
    o = nc.dram_tensor("o", [NLOC, D], f32, kind="ExternalOutput")

    DT = D // 128         # 8 d-tiles
    NQ = NLOC // 1024     # 2 column-halves of qT
    MQ = N // 1024        # 4 column-quarters of kT
    NCH = NLOC // N_CHUNK  # 8 main-loop chunks
    MT = N // M_TILE      # 32 m tiles
    VG = 8                # v row-groups of 512
    VPG = N // VG // 128  # 4 m-tiles per v group

    with TileContext(nc) as tc:
        with tc.tile_pool(name="singles", bufs=1) as singles, \
             tc.tile_pool(name="stream", bufs=20) as stream, \
             tc.tile_pool(name="vpool", bufs=VG) as vpool, \
             tc.tile_pool(name="expp", bufs=6) as expp, \
             tc.tile_pool(name="outp", bufs=3) as outp, \
             tc.tile_pool(name="rpool", bufs=4) as rpool, \
             tc.tile_pool(name="pacc", bufs=4, space="PSUM") as pacc, \
             tc.tile_pool(name="pscore", bufs=3, space="PSUM") as pscore, \
             tc.tile_pool(name="psums", bufs=1, space="PSUM") as psums:

            # ---- constants / projection weights ----
            wu_sb = singles.tile([128, DT, R], f32r, tag="wu")
            nc.sync.dma_start(out=wu_sb, in_=wu.rearrange("(t p) r -> p t r", p=128))
            wv_sb = singles.tile([128, DT, R], f32r, tag="wv")
            nc.sync.dma_start(out=wv_sb, in_=wv.rearrange("(t p) r -> p t r", p=128))
            ones = singles.tile([128, 2], f16, tag="ones")
            nc.vector.memset(ones, 1.0)

            uT = singles.tile([R, NLOC], f32r, tag="uT")
            vpT = singles.tile([R, N], f32r, tag="vpT")

            # ---- phase 1a: uT = Wu.T @ q  (per d-tile: wu_sb[:,t,:].T @ qT_t)
            def load_qt(h):
                tiles = []
                for t in range(DT):
                    tile = stream.tile([128, 1024], f32r, tag="stream",
                                       name=f"qt{h}_{t}")
                    nc.sync.dma_start(
                        out=tile, in_=qT[t * 128:(t + 1) * 128,
                                         h * 1024:(h + 1) * 1024])
                    tiles.append(tile)
                return tiles

            qt = {}
            for t, tile in enumerate(load_qt(0)):
                qt[(t, 0)] = tile
            def u_chunk(c):
                h, off = c // 2, (c % 2) * 512
                pu = pscore.tile([R, 512], f32, tag="scores", name=f"pu{c}")
                for t in range(DT):
                    nc.tensor.matmul(pu, lhsT=wu_sb[:, t, :],
                                     rhs=qt[(t, h)][:, off:off + 512],
                                     start=(t == 0), stop=(t == DT - 1))
                nc.vector.tensor_copy(out=uT[:, c * 512:(c + 1) * 512], in_=pu)

            for c in (0, 1):
                u_chunk(c)

            # ---- v tiles, interleaved with kT quarters so neither starves
            v_sb = [None] * VG

            def load_v(g):
                vt = vpool.tile([128, VPG, D], f16, tag="v", name=f"v{g}")
                nc.sync.dma_start(
                    out=vt, in_=v[g * 512:(g + 1) * 512, :].rearrange(
                        "(t p) d -> p t d", p=128))
                v_sb[g] = vt

            load_v(0)
            load_v(1)

            # ---- phase 1b: vpT = Wv.T @ k
            for qtr in range(MQ):
                kt = []
                for t in range(DT):
                    tile = stream.tile([128, 1024], f32r, tag="stream")
                    nc.sync.dma_start(
                        out=tile, in_=kT[t * 128:(t + 1) * 128,
                                         qtr * 1024:(qtr + 1) * 1024])
                    kt.append(tile)
                if qtr < 3:
                    load_v(2 + 2 * qtr)
                    load_v(3 + 2 * qtr)
                for c2 in range(2):
                    pv = pscore.tile([R, 512], f32, tag="scores")
                    for t in range(DT):
                        nc.tensor.matmul(pv, lhsT=wv_sb[:, t, :],
                                         rhs=kt[t][:, c2 * 512:c2 * 512 + 512],
                                         start=(t == 0), stop=(t == DT - 1))
                    off = qtr * 1024 + c2 * 512
                    nc.vector.tensor_copy(out=vpT[:, off:off + 512], in_=pv)

            for t, tile in enumerate(load_qt(1)):
                qt[(t, 1)] = tile
            for c in (2, 3):
                u_chunk(c)

            # ---- phase 2: flash-style scores/softmax/AV ----
            # software-pipelined: scores/exp for m-tile mt+1 are issued before
            # the AV matmuls of m-tile mt, so ScalarE exp latency hides under
            # the previous tile's AV work on the PE.
            for ch in range(NCH):
                accs = [pacc.tile([128, D_HALF], f32, tag="acc", name=f"acc{ch}_{i}")
                        for i in range(4)]
                # both sums accumulators share one bank: start=True clears
                # has_written bank-wide, so ONLY sums[0]'s first matmul carries
                # start=True (issued before any other write to the bank); the
                # cleared has_written makes sums[1]'s first start=False matmul
                # overwrite rather than accumulate stale data
                sums_t = psums.tile([128, 4], f32, tag="sums", name=f"sum{ch}")
                sums = [sums_t[:, 0:2], sums_t[:, 2:4]]

                def scores_exp(mt):
                    ps = pscore.tile([128, N_CHUNK], f32, tag="scores",
                                     name=f"ps{ch}_{mt}")
                    nc.tensor.matmul(
                        ps, lhsT=vpT[:, mt * 128:(mt + 1) * 128],
                        rhs=uT[:, ch * N_CHUNK:(ch + 1) * N_CHUNK],
                        start=True, stop=True)
                    ex = expp.tile([128, N_CHUNK], f16, tag="ex",
                                   name=f"ex{ch}_{mt}")
                    nc.scalar.activation(out=ex, in_=ps, func=EXP, scale=RSCALE)
                    return ex

                ex_q = [scores_exp(0), scores_exp(1)]
                for mt in range(MT):
                    ex = ex_q.pop(0)
                    if mt + 2 < MT:
                        ex_q.append(scores_exp(mt + 2))
                    g, tg = mt // VPG, mt % VPG
                    first, last = (mt == 0), (mt == MT - 1)
                    for j in range(2):
                        lhs = ex[:, j * 128:(j + 1) * 128]
                        nc.tensor.matmul(accs[2 * j], lhsT=lhs,
                                         rhs=v_sb[g][:, tg, 0:D_HALF],
                                         start=first, stop=last)
                        nc.tensor.matmul(accs[2 * j + 1], lhsT=lhs,
                                         rhs=v_sb[g][:, tg, D_HALF:D],
                                         start=first, stop=last)
                        nc.tensor.matmul(sums[j], lhsT=lhs, rhs=ones,
                                         start=(first and j == 0), stop=last,
                                         skip_group_check=True)
                # normalize on DVE (keeps ScalarE free for next chunk's exp)
                for j in range(2):
                    rc = rpool.tile([128, 1], f32, tag="rc", name=f"rc{ch}_{j}")
                    nc.vector.reciprocal(rc, sums[j][:, 0:1])
                    ob = outp.tile([128, D], f32, tag="ob", name=f"ob{ch}_{j}")
                    nc.vector.tensor_scalar_mul(ob[:, 0:D_HALF], accs[2 * j], rc)
                    nc.vector.tensor_scalar_mul(ob[:, D_HALF:D], accs[2 * j + 1], rc)
                    row = ch * N_CHUNK + j * 128
                    nc.sync.dma_start(out=o[row:row + 128, :], in_=ob)

    nc.finalize()
    return nc


def kernel(q, k, v, Wu, Wv):
    global LAST_RESULT
    from concourse import bass_utils

    nc = _build()

    kTs = [np.ascontiguousarray(k[b].T) for b in range(B)]
    vs = [np.ascontiguousarray(v[b]).astype(np.float16) for b in range(B)]
    in_maps = []
    for core in range(8):
        b, h = core // 2, core % 2
        in_maps.append({
            "qT": np.ascontiguousarray(q[b].T[:, h * NLOC:(h + 1) * NLOC]),
            "kT": kTs[b],
            "v": vs[b],
            "wu": np.ascontiguousarray(Wu),
            "wv": np.ascontiguousarray(Wv),
        })

    res = bass_utils.run_bass_kernel_spmd(nc, in_maps, core_ids=list(range(8)))
    LAST_RESULT = res

    out = np.empty((B, N, D), dtype=np.float32)
    for core in range(8):
        b, h = core // 2, core % 2
        out[b, h * NLOC:(h + 1) * NLOC, :] = res.results[core]["o"]
    return out



# revision 18
# speedup vs baseline: 1.5842x; 1.5842x over previous
"""Low-rank attention kernel for Trainium2, distributed over 8 NeuronCores.

Math (per batch b):
    u  = q @ Wu            [N, R]
    vp = k @ Wv            [N, R]
    S  = u @ vp.T / sqrt(R)
    out = softmax(S) @ v   [N, D]

Shapes: B=4, N=4096, D=1024, R=32.

Sharding: data-parallel over batch x row-halves -> 8 shards. Core c handles
batch b = c // 2, rows [h*2048, (h+1)*2048) with h = c % 2. Each core gets its
q-shard and the full k/v for its batch, all in float16 (halves HBM traffic vs
f32; end-to-end max rel err ~1e-3 vs the 2e-2 budget).

Per-core device kernel:
  1. uT[R, 2048]  = sum_d Wu[d, :].T qT[d, :]   (K=128 d-tiles, PSUM accum)
     vpT[R, 4096] = sum_d Wv[d, :].T kT[d, :]
     vp quarters and the late u chunks are interleaved into the flash stream
     so the PE never waits on the tail of the kT/qT DMA streams.
  2. one continuous flash pipeline over all (chunk, m-pair) steps:
       ps[m256-pair, n256] in one PSUM bank (two 128-col matmuls)
       ex = Exp(ps / sqrt(R))          one ScalarE instr per pair (f16 out)
       sum_acc[n128, 1]    += ex_tile.T @ ones      (issued before the AV
       out_acc[n128, d512] += ex_tile.T @ v_tile     matmuls so the final
                                                     reciprocal starts early)
     scores/exp for the next chunk are issued before the current chunk's AV
     tail, so chunk boundaries cost no exp-latency bubble.
     out = out_acc * (1 / sum_acc): recips on DVE, the [128,512] muls split
     across DVE + ScalarE (Copy shares the exp act-func table), o streamed
     out in f16 halves right behind each mul.
"""

import numpy as np

B, N, D, R = 4, 4096, 1024, 32
NLOC = N // 2            # rows per core
RSCALE = float(1.0 / np.sqrt(np.float32(R)))

N_CHUNK = 256            # rows of scores computed per PSUM round
D_HALF = 512             # PSUM bank width in fp32

LAST_RESULT = None       # test.py reads exec_time_ns etc. from here
LAST_NC = None           # built Bass module, for test.py's bench loop
LAST_IN_MAPS = None      # per-core input maps, for test.py's bench loop


def _build():
    from concourse import bacc, mybir
    from concourse.tile import TileContext

    f32 = mybir.dt.float32
    f16 = mybir.dt.float16
    EXP = mybir.ActivationFunctionType.Exp

    nc = bacc.Bacc("TRN2", target_bir_lowering=False)

    qT = nc.dram_tensor("qT", [D, NLOC], f16, kind="ExternalInput")
    kT = nc.dram_tensor("kT", [D, N], f16, kind="ExternalInput")
    v = nc.dram_tensor("v", [N, D], f16, kind="ExternalInput")
    wu = nc.dram_tensor("wu", [D, R], f16, kind="ExternalInput")
    wv = nc.dram_tensor("wv", [D, R], f16, kind="ExternalInput")
    o = nc.dram_tensor("o", [NLOC, D], f16, kind="ExternalOutput")

    DT = D // 128         # 8 d-tiles
    NCH = NLOC // N_CHUNK  # 8 flash chunks
    MT = N // 128         # 32 m tiles
    NP = MT // 2          # 16 m-tile pairs per chunk
    GP = NCH * NP         # 128 global pairs
    VG = 8                # v row-groups of 512
    VPG = N // VG // 128  # 4 m-tiles per v group

    with TileContext(nc) as tc:
        with tc.tile_pool(name="singles", bufs=1) as singles, \
             tc.tile_pool(name="stream", bufs=10) as stream, \
             tc.tile_pool(name="vpool", bufs=VG) as vpool, \
             tc.tile_pool(name="expp", bufs=4) as expp, \
             tc.tile_pool(name="outp", bufs=4) as outp, \
             tc.tile_pool(name="rpool", bufs=4) as rpool, \
             tc.tile_pool(name="pacc", bufs=4, space="PSUM") as pacc, \
             tc.tile_pool(name="pscore", bufs=2, space="PSUM") as pscore, \
             tc.tile_pool(name="pproj", bufs=1, space="PSUM") as pproj, \
             tc.tile_pool(name="psums", bufs=1, space="PSUM") as psums:

            # ---- constants / projection weights ----
            wu_sb = singles.tile([128, DT, R], f16, tag="wu")
            nc.sync.dma_start(out=wu_sb, in_=wu.rearrange("(t p) r -> p t r", p=128))
            wv_sb = singles.tile([128, DT, R], f16, tag="wv")
            nc.sync.dma_start(out=wv_sb, in_=wv.rearrange("(t p) r -> p t r", p=128))
            ones = singles.tile([128, 2], f16, tag="ones")
            nc.vector.memset(ones, 1.0)
            # dummy exp at t~0: forces the ScalarE act-func table DMA
            # (~2.7us) to happen under the input-DMA shadow, not on the
            # first real exp of the flash loop
            warm = singles.tile([128, 2], f16, tag="warm")
            nc.scalar.activation(out=warm, in_=ones, func=EXP, scale=1.0)

            uT = singles.tile([R, NLOC], f16, tag="uT")
            vpT = singles.tile([R, N], f16, tag="vpT")

            # ---- DMA issue order = approximate arrival order ----
            # kT/qT loaded as [128, 8, 512] column-halves (one descriptor per
            # 512-col half across all 8 d-tiles: few, fat DMAs -> the SP
            # queue isn't descriptor-issue-bound). v groups interleaved in
            # the order the flash loop consumes them; qT h1 last (needed
            # from chunk 4, ~150us in).
            kt_tiles = {}

            def load_kt(qtr, c2, parts=1):
                tile = stream.tile([128, DT, 512], f16, tag="stream",
                                   name=f"kt{qtr}_{c2}")
                col = qtr * 1024 + c2 * 512
                dt2 = DT // parts
                for s in range(parts):
                    nc.sync.dma_start(
                        out=tile[:, s * dt2:(s + 1) * dt2, :],
                        in_=kT[s * dt2 * 128:(s + 1) * dt2 * 128,
                               col:col + 512].rearrange(
                            "(t p) c -> p t c", p=128))
                kt_tiles[(qtr, c2)] = tile

            qt_tiles = {}

            def load_qt(h, c2, parts=1):
                tile = stream.tile([128, DT, 512], f16, tag="stream",
                                   name=f"qt{h}_{c2}")
                col = h * 1024 + c2 * 512
                dt2 = DT // parts
                for s in range(parts):
                    nc.sync.dma_start(
                        out=tile[:, s * dt2:(s + 1) * dt2, :],
                        in_=qT[s * dt2 * 128:(s + 1) * dt2 * 128,
                               col:col + 512].rearrange(
                            "(t p) c -> p t c", p=128))
                qt_tiles[(h, c2)] = tile

            v_sb = [None] * VG

            def load_v(g):
                vt = vpool.tile([128, VPG, D], f16, tag="v", name=f"v{g}")
                nc.sync.dma_start(
                    out=vt, in_=v[g * 512:(g + 1) * 512, :].rearrange(
                        "(t p) d -> p t d", p=128))
                v_sb[g] = vt

            load_kt(0, 0, parts=4)
            load_kt(0, 1, parts=2)
            load_qt(0, 0, parts=2)
            load_qt(0, 1, parts=2)
            load_v(0)
            load_v(1)
            load_kt(1, 0)
            load_kt(1, 1)
            load_v(2)
            load_v(3)
            load_kt(2, 0)
            load_kt(2, 1)
            load_v(4)
            load_v(5)
            load_kt(3, 0)
            load_kt(3, 1)
            load_v(6)
            load_v(7)
            load_qt(1, 0)
            load_qt(1, 1)

            # ---- projection emitters (PE accum + DVE copy out of PSUM) ----
            def proj_512(w_sb, tiles, key, out_ap, name):
                pp = pproj.tile([R, 512], f32, tag="proj", name=name)
                for t in range(DT):
                    nc.tensor.matmul(pp, lhsT=w_sb[:, t, :],
                                     rhs=tiles[key][:, t, :],
                                     start=(t == 0), stop=(t == DT - 1))
                nc.vector.tensor_copy(out=out_ap, in_=pp)

            def u_chunk(c):
                h, c2 = c // 2, c % 2
                proj_512(wu_sb, qt_tiles, (h, c2),
                         uT[:, c * 512:(c + 1) * 512], f"pu{c}")

            def vp_quarter(qtr):
                for c2 in range(2):
                    off = qtr * 1024 + c2 * 512
                    proj_512(wv_sb, kt_tiles, (qtr, c2),
                             vpT[:, off:off + 512], f"pv{qtr}_{c2}")



            # ---- continuous flash pipeline over 128 global pairs ----
            # inject: global pair index -> thunk emitted before that pair's
            # scores are issued (slots projection work into the in-order PE
            # stream exactly where its inputs have arrived).
            inject = {2: lambda: vp_quarter(1),
                      6: lambda: vp_quarter(2),
                      10: lambda: vp_quarter(3),
                      34: lambda: u_chunk(2),
                      38: lambda: u_chunk(3)}

            def scores_exp(g):
                if g in inject:
                    inject[g]()
                ch = g // NP
                ps = pscore.tile([128, 2, N_CHUNK], f32, tag="scores",
                                 name=f"ps{g}")
                for i in range(2):
                    mt = 2 * (g % NP) + i
                    nc.tensor.matmul(
                        ps[:, i, :],
                        lhsT=vpT[:, mt * 128:(mt + 1) * 128],
                        rhs=uT[:, ch * N_CHUNK:(ch + 1) * N_CHUNK],
                        start=(i == 0), stop=(i == 1),
                        skip_group_check=True)
                ex = expp.tile([128, 2, N_CHUNK], f16, tag="ex", name=f"ex{g}")
                nc.scalar.activation(out=ex, in_=ps, func=EXP, scale=RSCALE)
                return ex

            accs = None
            sums = None

            def normalize(ch, accs, sums):
                # recips on DVE; [128,512] muls split DVE/ScalarE; each half's
                # DMA issued off the SP queue (gpsimd SWDGE for the DVE half,
                # the Activation HWDGE for its own half) so output descriptors
                # never serialize behind input loads.
                for j in range(2):
                    rc = rpool.tile([128, 1], f32, tag="rc", name=f"rc{ch}_{j}")
                    nc.vector.reciprocal(rc, sums[j][:, 0:1])
                    ob = outp.tile([128, D], f16, tag="ob", name=f"ob{ch}_{j}")
                    row = ch * N_CHUNK + j * 128
                    nc.vector.tensor_scalar_mul(ob[:, 0:D_HALF], accs[2 * j], rc)
                    nc.gpsimd.dma_start(out=o[row:row + 128, 0:D_HALF],
                                        in_=ob[:, 0:D_HALF])
                    nc.scalar.mul(ob[:, D_HALF:D], accs[2 * j + 1], rc)
                    nc.scalar.dma_start(out=o[row:row + 128, D_HALF:D],
                                        in_=ob[:, D_HALF:D])

            # prologue: projections feeding chunk 0, then the pipeline
            vp_quarter(0)
            u_chunk(0)
            u_chunk(1)
            ex_q = [scores_exp(0), scores_exp(1)]
            for g in range(GP):
                ch, p = g // NP, g % NP
                if p == 0:
                    accs = [pacc.tile([128, D_HALF], f32, tag="acc",
                                      name=f"acc{ch}_{i}") for i in range(4)]
                    sums_t = psums.tile([128, 4], f32, tag="sums",
                                        name=f"sum{ch}")
                    sums = [sums_t[:, 0:2], sums_t[:, 2:4]]
                ex = ex_q.pop(0)
                if g + 2 < GP:
                    ex_q.append(scores_exp(g + 2))
                first_pair, last_pair = (p == 0), (p == NP - 1)
                for i in range(2):
                    mt = 2 * p + i
                    grp, tg = mt // VPG, mt % VPG
                    first, last = (first_pair and i == 0), \
                        (last_pair and i == 1)
                    for j in range(2):
                        lhs = ex[:, i, j * 128:(j + 1) * 128]
                        nc.tensor.matmul(sums[j], lhsT=lhs, rhs=ones,
                                         start=(first and j == 0), stop=last,
                                         skip_group_check=True)
                    for j in range(2):
                        lhs = ex[:, i, j * 128:(j + 1) * 128]
                        nc.tensor.matmul(accs[2 * j], lhsT=lhs,
                                         rhs=v_sb[grp][:, tg, 0:D_HALF],
                                         start=first, stop=last)
                        nc.tensor.matmul(accs[2 * j + 1], lhsT=lhs,
                                         rhs=v_sb[grp][:, tg, D_HALF:D],
                                         start=first, stop=last)
                if last_pair:
                    normalize(ch, accs, sums)

    nc.finalize()
    return nc


def kernel(q, k, v, Wu, Wv):
    global LAST_RESULT, LAST_NC, LAST_IN_MAPS
    from concourse import bass_utils

    nc = _build()
    LAST_NC = nc

    kTs = [np.ascontiguousarray(k[b].T.astype(np.float16)) for b in range(B)]
    vs = [np.ascontiguousarray(v[b]).astype(np.float16) for b in range(B)]
    wu16 = np.ascontiguousarray(Wu.astype(np.float16))
    wv16 = np.ascontiguousarray(Wv.astype(np.float16))
    in_maps = []
    for core in range(8):
        b, h = core // 2, core % 2
        in_maps.append({
            "qT": np.ascontiguousarray(
                q[b].T[:, h * NLOC:(h + 1) * NLOC].astype(np.float16)),
            "kT": kTs[b],
            "v": vs[b],
            "wu": wu16,
            "wv": wv16,
        })
    LAST_IN_MAPS = in_maps

    res = bass_utils.run_bass_kernel_spmd(nc, in_maps, core_ids=list(range(8)))
    LAST_RESULT = res

    out = np.empty((B, N, D), dtype=np.float32)
    for core in range(8):
        b, h = core // 2, core % 2
        out[b, h * NLOC:(h + 1) * NLOC, :] = \
            res.results[core]["o"].astype(np.float32)
    return out


# revision 22
# speedup vs baseline: 1.5897x; 1.0035x over previous
"""Low-rank attention kernel for Trainium2, distributed over 8 NeuronCores.

Math (per batch b):
    u  = q @ Wu            [N, R]
    vp = k @ Wv            [N, R]
    S  = u @ vp.T / sqrt(R)
    out = softmax(S) @ v   [N, D]

Shapes: B=4, N=4096, D=1024, R=32.

Sharding: data-parallel over batch x row-halves -> 8 shards. Core c handles
batch b = c // 2, rows [h*2048, (h+1)*2048) with h = c % 2. Each core gets its
q-shard and the full k/v for its batch, all in float16 (halves HBM traffic vs
f32; end-to-end max rel err ~1e-3 vs the 2e-2 budget).

Per-core device kernel (PE busy ~267us of ~285us CoreSim total; AV matmul
is 218us of it and runs at the f16 1-cycle/row roofline — fp8/DoubleRow was
measured numerically out of budget for the 2e-2 gate):
  1. uT[R, 2048]  = sum_d Wu[d, :].T qT[d, :]   (K=128 d-tiles, PSUM accum)
     vpT[R, 4096] = sum_d Wv[d, :].T kT[d, :]
     vp quarters and the late u chunks are interleaved into the flash stream
     so the PE never waits on the tail of the kT/qT DMA streams.
  2. one continuous flash pipeline over all (chunk, m-pair) steps:
       ps[m256-pair, n256] in one PSUM bank (two 128-col matmuls)
       ex = Exp(ps / sqrt(R))          one ScalarE instr per pair (f16 out)
       sum_acc[n128, 1]    += ex_tile.T @ ones      (issued before the AV
       out_acc[n128, d512] += ex_tile.T @ v_tile     matmuls so the final
                                                     reciprocal starts early)
     scores/exp for the next chunk are issued before the current chunk's AV
     tail, so chunk boundaries cost no exp-latency bubble.
     out = out_acc * (1 / sum_acc): recips on DVE, the [128,512] muls split
     across DVE + ScalarE (Copy shares the exp act-func table), o streamed
     out in f16 halves right behind each mul.

PSUM budget (8 banks): 4 AV accumulators + 2 paired-score banks + 1
projection accumulator + 1 row-sums bank.
"""

import numpy as np

B, N, D, R = 4, 4096, 1024, 32
NLOC = N // 2            # rows per core
RSCALE = float(1.0 / np.sqrt(np.float32(R)))

N_CHUNK = 256            # rows of scores computed per PSUM round
D_HALF = 512             # PSUM bank width in fp32

LAST_RESULT = None       # test.py reads exec_time_ns etc. from here
LAST_NC = None           # built Bass module, for test.py's bench loop
LAST_IN_MAPS = None      # per-core input maps, for test.py's bench loop


def _build():
    from concourse import bacc, mybir
    from concourse.tile import TileContext

    f32 = mybir.dt.float32
    f16 = mybir.dt.float16
    EXP = mybir.ActivationFunctionType.Exp

    nc = bacc.Bacc("TRN2", target_bir_lowering=False)

    qT = nc.dram_tensor("qT", [D, NLOC], f16, kind="ExternalInput")
    kT = nc.dram_tensor("kT", [D, N], f16, kind="ExternalInput")
    v = nc.dram_tensor("v", [N, D], f16, kind="ExternalInput")
    wu = nc.dram_tensor("wu", [D, R], f16, kind="ExternalInput")
    wv = nc.dram_tensor("wv", [D, R], f16, kind="ExternalInput")
    o = nc.dram_tensor("o", [NLOC, D], f16, kind="ExternalOutput")

    DT = D // 128         # 8 d-tiles
    NCH = NLOC // N_CHUNK  # 8 flash chunks
    MT = N // 128         # 32 m tiles
    NP = MT // 2          # 16 m-tile pairs per chunk
    GP = NCH * NP         # 128 global pairs
    VG = 16               # v row-groups of 256
    VPG = N // VG // 128  # 2 m-tiles per v group

    with TileContext(nc) as tc:
        with tc.tile_pool(name="singles", bufs=1) as singles, \
             tc.tile_pool(name="stream", bufs=10) as stream, \
             tc.tile_pool(name="vpool", bufs=VG) as vpool, \
             tc.tile_pool(name="expp", bufs=4) as expp, \
             tc.tile_pool(name="outp", bufs=4) as outp, \
             tc.tile_pool(name="rpool", bufs=4) as rpool, \
             tc.tile_pool(name="pacc", bufs=4, space="PSUM") as pacc, \
             tc.tile_pool(name="pscore", bufs=2, space="PSUM") as pscore, \
             tc.tile_pool(name="pproj", bufs=1, space="PSUM") as pproj, \
             tc.tile_pool(name="psums", bufs=1, space="PSUM") as psums:

            # ---- constants / projection weights ----
            wu_sb = singles.tile([128, DT, R], f16, tag="wu")
            nc.sync.dma_start(out=wu_sb, in_=wu.rearrange("(t p) r -> p t r", p=128))
            wv_sb = singles.tile([128, DT, R], f16, tag="wv")
            nc.sync.dma_start(out=wv_sb, in_=wv.rearrange("(t p) r -> p t r", p=128))
            ones = singles.tile([128, 2], f16, tag="ones")
            nc.vector.memset(ones, 1.0)
            # dummy exp at t~0: forces the ScalarE act-func table DMA
            # (~2.7us) to happen under the input-DMA shadow, not on the
            # first real exp of the flash loop
            warm = singles.tile([128, 2], f16, tag="warm")
            nc.scalar.activation(out=warm, in_=ones, func=EXP, scale=1.0)

            uT = singles.tile([R, NLOC], f16, tag="uT")
            vpT = singles.tile([R, N], f16, tag="vpT")

            # ---- DMA issue order = approximate arrival order ----
            # kT/qT loaded as [128, 8, 512] column-halves (one descriptor per
            # 512-col half across all 8 d-tiles: few, fat DMAs -> the SP
            # queue isn't descriptor-issue-bound). v groups interleaved in
            # the order the flash loop consumes them; qT h1 last (needed
            # from chunk 4, ~150us in).
            kt_tiles = {}

            def load_kt(qtr, c2, parts=1):
                tile = stream.tile([128, DT, 512], f16, tag="stream",
                                   name=f"kt{qtr}_{c2}")
                col = qtr * 1024 + c2 * 512
                dt2 = DT // parts
                for s in range(parts):
                    nc.sync.dma_start(
                        out=tile[:, s * dt2:(s + 1) * dt2, :],
                        in_=kT[s * dt2 * 128:(s + 1) * dt2 * 128,
                               col:col + 512].rearrange(
                            "(t p) c -> p t c", p=128))
                kt_tiles[(qtr, c2)] = tile

            qt_tiles = {}

            def load_qt(h, c2, parts=1):
                tile = stream.tile([128, DT, 512], f16, tag="stream",
                                   name=f"qt{h}_{c2}")
                col = h * 1024 + c2 * 512
                dt2 = DT // parts
                for s in range(parts):
                    nc.sync.dma_start(
                        out=tile[:, s * dt2:(s + 1) * dt2, :],
                        in_=qT[s * dt2 * 128:(s + 1) * dt2 * 128,
                               col:col + 512].rearrange(
                            "(t p) c -> p t c", p=128))
                qt_tiles[(h, c2)] = tile

            v_sb = [None] * VG

            def load_v(g):
                rows = VPG * 128
                vt = vpool.tile([128, VPG, D], f16, tag="v", name=f"v{g}")
                nc.sync.dma_start(
                    out=vt, in_=v[g * rows:(g + 1) * rows, :].rearrange(
                        "(t p) d -> p t d", p=128))
                v_sb[g] = vt

            load_kt(0, 0, parts=4)
            load_kt(0, 1, parts=2)
            load_qt(0, 0, parts=2)
            load_qt(0, 1, parts=2)
            load_v(0)
            load_v(1)
            load_v(2)
            load_v(3)
            load_kt(1, 0)
            load_kt(1, 1)
            load_v(4)
            load_v(5)
            load_v(6)
            load_v(7)
            load_kt(2, 0)
            load_kt(2, 1)
            load_v(8)
            load_v(9)
            load_v(10)
            load_v(11)
            load_kt(3, 0)
            load_kt(3, 1)
            load_v(12)
            load_v(13)
            load_v(14)
            load_v(15)
            load_qt(1, 0)
            load_qt(1, 1)

            # ---- projection emitters (PE accum + DVE copy out of PSUM) ----
            def proj_512(w_sb, tiles, key, out_ap, name):
                pp = pproj.tile([R, 512], f32, tag="proj", name=name)
                for t in range(DT):
                    nc.tensor.matmul(pp, lhsT=w_sb[:, t, :],
                                     rhs=tiles[key][:, t, :],
                                     start=(t == 0), stop=(t == DT - 1))
                nc.vector.tensor_copy(out=out_ap, in_=pp)

            def u_chunk(c):
                h, c2 = c // 2, c % 2
                proj_512(wu_sb, qt_tiles, (h, c2),
                         uT[:, c * 512:(c + 1) * 512], f"pu{c}")

            def vp_quarter(qtr):
                for c2 in range(2):
                    off = qtr * 1024 + c2 * 512
                    proj_512(wv_sb, kt_tiles, (qtr, c2),
                             vpT[:, off:off + 512], f"pv{qtr}_{c2}")



            # ---- continuous flash pipeline over 128 global pairs ----
            # inject: global pair index -> thunk emitted before that pair's
            # scores are issued (slots projection work into the in-order PE
            # stream exactly where its inputs have arrived).
            inject = {2: lambda: vp_quarter(1),
                      6: lambda: vp_quarter(2),
                      10: lambda: vp_quarter(3),
                      34: lambda: u_chunk(2),
                      38: lambda: u_chunk(3)}

            def scores_exp(g):
                if g in inject:
                    inject[g]()
                ch = g // NP
                ps = pscore.tile([128, 2, N_CHUNK], f32, tag="scores",
                                 name=f"ps{g}")
                for i in range(2):
                    mt = 2 * (g % NP) + i
                    nc.tensor.matmul(
                        ps[:, i, :],
                        lhsT=vpT[:, mt * 128:(mt + 1) * 128],
                        rhs=uT[:, ch * N_CHUNK:(ch + 1) * N_CHUNK],
                        start=(i == 0), stop=(i == 1),
                        skip_group_check=True)
                ex = expp.tile([128, 2, N_CHUNK], f16, tag="ex", name=f"ex{g}")
                nc.scalar.activation(out=ex, in_=ps, func=EXP, scale=RSCALE)
                return ex

            accs = None
            sums = None

            def normalize(ch, accs, sums):
                # recips on DVE; [128,512] muls split DVE/ScalarE; the two
                # half-DMAs issue from different HWDGE queues (SP for the DVE
                # half, Activation for its own half) so output descriptors
                # don't serialize on one engine at the kernel tail.
                for j in range(2):
                    rc = rpool.tile([128, 1], f32, tag="rc", name=f"rc{ch}_{j}")
                    nc.vector.reciprocal(rc, sums[j][:, 0:1])
                    ob = outp.tile([128, D], f16, tag="ob", name=f"ob{ch}_{j}")
                    row = ch * N_CHUNK + j * 128
                    nc.vector.tensor_scalar_mul(ob[:, 0:D_HALF], accs[2 * j], rc)
                    nc.sync.dma_start(out=o[row:row + 128, 0:D_HALF],
                                      in_=ob[:, 0:D_HALF])
                    nc.scalar.mul(ob[:, D_HALF:D], accs[2 * j + 1], rc)
                    nc.scalar.dma_start(out=o[row:row + 128, D_HALF:D],
                                        in_=ob[:, D_HALF:D])

            # prologue: projections feeding chunk 0, then the pipeline
            vp_quarter(0)
            u_chunk(0)
            u_chunk(1)
            ex_q = [scores_exp(0), scores_exp(1)]
            for g in range(GP):
                ch, p = g // NP, g % NP
                if p == 0:
                    accs = [pacc.tile([128, D_HALF], f32, tag="acc",
                                      name=f"acc{ch}_{i}") for i in range(4)]
                    sums_t = psums.tile([128, 4], f32, tag="sums",
                                        name=f"sum{ch}")
                    sums = [sums_t[:, 0:2], sums_t[:, 2:4]]
                ex = ex_q.pop(0)
                if g + 2 < GP:
                    ex_q.append(scores_exp(g + 2))
                first_pair, last_pair = (p == 0), (p == NP - 1)
                for i in range(2):
                    mt = 2 * p + i
                    grp, tg = mt // VPG, mt % VPG
                    first, last = (first_pair and i == 0), \
                        (last_pair and i == 1)
                    for j in range(2):
                        lhs = ex[:, i, j * 128:(j + 1) * 128]
                        nc.tensor.matmul(sums[j], lhsT=lhs, rhs=ones,
                                         start=(first and j == 0), stop=last,
                                         skip_group_check=True)
                    for j in range(2):
                        lhs = ex[:, i, j * 128:(j + 1) * 128]
                        nc.tensor.matmul(accs[2 * j], lhsT=lhs,
                                         rhs=v_sb[grp][:, tg, 0:D_HALF],
                                         start=first, stop=last)
                        nc.tensor.matmul(accs[2 * j + 1], lhsT=lhs,
                                         rhs=v_sb[grp][:, tg, D_HALF:D],
                                         start=first, stop=last)
                if last_pair:
                    normalize(ch, accs, sums)

    nc.finalize()
    return nc


def kernel(q, k, v, Wu, Wv):
    global LAST_RESULT, LAST_NC, LAST_IN_MAPS
    from concourse import bass_utils

    nc = _build()
    LAST_NC = nc

    kTs = [np.ascontiguousarray(k[b].T.astype(np.float16)) for b in range(B)]
    vs = [np.ascontiguousarray(v[b]).astype(np.float16) for b in range(B)]
    wu16 = np.ascontiguousarray(Wu.astype(np.float16))
    wv16 = np.ascontiguousarray(Wv.astype(np.float16))
    in_maps = []
    for core in range(8):
        b, h = core // 2, core % 2
        in_maps.append({
            "qT": np.ascontiguousarray(
                q[b].T[:, h * NLOC:(h + 1) * NLOC].astype(np.float16)),
            "kT": kTs[b],
            "v": vs[b],
            "wu": wu16,
            "wv": wv16,
        })
    LAST_IN_MAPS = in_maps

    res = bass_utils.run_bass_kernel_spmd(nc, in_maps, core_ids=list(range(8)))
    LAST_RESULT = res

    out = np.empty((B, N, D), dtype=np.float32)
    for core in range(8):
        b, h = core // 2, core % 2
        out[b, h * NLOC:(h + 1) * NLOC, :] = \
            res.results[core]["o"].astype(np.float32)
    return out
